# revision 47
# baseline (speedup 1.0000x reference)
"""Trainium2 Bass kernel for the 2-player masked LSTM scan.

Reference semantics (T=128 steps, B=256 batch, C=1024 in, H=1024 hidden):
  per step t, batch b: the active player's (c,h) (selected by main[t,b]) runs
  one LSTM cell z = x@Wi + h@Wh + b with fused i,f,g,o gates; the result is
  written back only to the active player's state, and both players' states are
  zeroed where done[t,b].

Key algorithmic idea: done/main are *inputs*, so the true dependency structure
is known on the host.  Each (b, segment, player) triple forms an independent
"chain" of positions; a position at depth d in its chain depends only on the
position at depth d-1.  With done ~ Bernoulli(0.5) per step, chains are short
(max depth ~9 for the target inputs), so the sequential scan of 128 steps
becomes ~9 dense "waves", each a full-batch matmul with no masking at all.

Host side: sort chains by length (desc), round-robin across the 8 cores, lay
positions out wave-major.  Because chain order is sorted by length, the chains
alive at depth d are exactly a prefix of those alive at depth d-1 - so wave d
reads a contiguous prefix of wave d-1's outputs: no gather needed on device.

Device: phase A computes zx = x@Wi (+ bias via DVE) for all positions, fusing
the full gate math for depth-0 positions (input state is zero); phase B runs
one matmul z = zx + h@Wh per wave plus the LSTM gate math.  bf16 matmuls with
fp32 PSUM accumulation; the carried cell state c stays fp32.

Scheduling: phase A emits the wave-1-feeding prefix tiles first, then the
wave-zx tiles, then the remaining depth-0 tiles, so wave 1's h-transpose and
zx loads complete long before the waves start (no phase-boundary PE stall).
Tail waves (one m-tile) keep all state in SBUF; h^T for the next wave is
built with PE transpose instructions (no xbar DMA on the critical path), the
zx injection runs as the PSUM-group opener so it executes during the previous
wave's drain, and per-half alternating PSUM tags let consecutive waves'
matmuls overlap each other's drains.
"""

import sys

sys.path.insert(0, "/opt/trn_rl_repo")

import numpy as np
import ml_dtypes

import concourse.bass as bass
import concourse.tile as tile
from concourse import bacc, mybir
from concourse.bass_utils import run_bass_kernel_spmd

BF16 = ml_dtypes.bfloat16
F8 = ml_dtypes.float8_e4m3
AF = mybir.ActivationFunctionType
DT = mybir.dt

NCORES = 8
H = 1024
CIN = 1024
G = 4 * H  # 4096 fused gate width
KT = CIN // 128  # 8 k-tiles for both Wi and Wh contractions


# ---------------------------------------------------------------------------
# Host-side schedule construction
# ---------------------------------------------------------------------------

def _build_schedule(done, main, T, B):
    """Chain decomposition of the (t, b) grid.

    Returns per-position (core, depth, rank) and the uniform padded wave
    geometry shared by all cores (SPMD requires identical programs).
    """
    done2 = done.reshape(T, B).astype(bool)
    main2 = main.reshape(T, B).astype(bool)

    seg = np.zeros((T, B), np.int64)
    if T > 1:
        seg[1:] = np.cumsum(done2[:-1], axis=0)
    player = main2.astype(np.int64)
    key = (np.arange(B)[None, :] * (T + 1) + seg) * 2 + player  # [T, B]
    flat_key = key.reshape(-1)  # position p = t*B + b
    order = np.argsort(flat_key, kind="stable")  # chain-major, t-ascending
    sorted_keys = flat_key[order]
    uk, first_idx, inv = np.unique(sorted_keys, return_index=True, return_inverse=True)
    chain_len = np.diff(np.append(first_idx, len(sorted_keys)))
    npos = T * B

    depth = np.empty(npos, np.int64)
    depth[order] = np.arange(npos) - first_idx[inv]
    chain_id = np.empty(npos, np.int64)
    chain_id[order] = inv

    n_chains = len(uk)
    chain_b = uk // (2 * (T + 1))
    chain_seg = (uk // 2) % (T + 1)
    chain_player = uk % 2

    chain_order = np.argsort(-chain_len, kind="stable")
    rank_of_chain = np.empty(n_chains, np.int64)
    rank_of_chain[chain_order] = np.arange(n_chains)
    core_of_chain = (rank_of_chain % NCORES).astype(np.int64)
    core_rank = rank_of_chain // NCORES

    D = int(chain_len.max())
    lens_sorted = np.sort(chain_len)
    N_d = np.array([n_chains - np.searchsorted(lens_sorted, d, side="right")
                    for d in range(D)], np.int64)
    U = np.ceil(N_d / NCORES).astype(np.int64)      # uniform per-core wave rows
    M = np.ceil(U / 128).astype(np.int64)           # padded wave m-tiles
    V = np.concatenate([[0], np.cumsum(U)])          # packed row offsets
    P = np.concatenate([[0], np.cumsum(M * 128)])    # padded row offsets

    return dict(
        depth=depth, chain_id=chain_id, core_of_chain=core_of_chain,
        core_rank=core_rank, chain_b=chain_b, chain_seg=chain_seg,
        chain_player=chain_player, D=D, U=U, M=M, V=V, P=P,
    )


def _prep_inputs(x, c1, h1, c2, h2, Wi, Wh, b, done, main):
    """Build per-core device input arrays + output scatter indices."""
    B = c1.shape[0]
    T = x.shape[0] // B
    sch = _build_schedule(np.asarray(done), np.asarray(main), T, B)
    D, U, M, V, P = sch["D"], sch["U"], sch["M"], sch["V"], sch["P"]

    zero_init = not (np.any(c1) or np.any(h1) or np.any(c2) or np.any(h2))

    packed_total = int(V[D])
    # compute tiles cover the packed rows only; the zx array is sized for the
    # waves' padded reads (rows past packed_total are never-consumed garbage)
    need = packed_total
    for d in range(1 if zero_init else 0, D):
        need = max(need, int(V[d]) + int(M[d]) * 128)
    Mzx = (packed_total + 127) // 128
    zx_row0 = int(V[1]) // 128 * 128 if (zero_init and D > 1) else 0
    zx_start_tile = zx_row0 // 128
    zx_rows = max(need - zx_row0, 128)

    depth = sch["depth"]; chain_id = sch["chain_id"]
    core_pos = sch["core_of_chain"][chain_id]
    packed_row = V[depth] + sch["core_rank"][chain_id]
    padded_row = P[depth] + sch["core_rank"][chain_id]

    x = np.ascontiguousarray(np.asarray(x, np.float32))
    xt_blocks = []
    for c in range(NCORES):
        sel = core_pos == c
        Xp = np.zeros((Mzx * 128, CIN), np.float32)
        Xp[packed_row[sel]] = x[sel]
        # lhsT block layout: [mt, p, k*128 + m] = Xp[mt*128+m, k*128+p]
        xt = Xp.reshape(Mzx, 128, KT, 128).transpose(0, 3, 2, 1).reshape(Mzx, 128, CIN)
        xt_blocks.append(np.ascontiguousarray(xt.astype(BF16)))

    # Wi in PE demand order: [half, k, p, g*512+c] = Wi[k*128+p, g*H+half*512+c]
    # so phase A's first m-tile consumes DMA chunks in arrival order.
    Wi_l = np.ascontiguousarray(
        np.asarray(Wi, np.float32).reshape(KT, 128, 4, 2, 512)
        .transpose(3, 0, 1, 2, 4).reshape(2, KT, 128, 2048).astype(BF16))
    # Wh in fp8 DoubleRow pair layout: [kp, p, i*G + n] = Wh[(2kp+i)*128+p, n]
    Wh_l = np.ascontiguousarray(
        np.asarray(Wh, np.float32).reshape(KT // 2, 2, 128, G)
        .transpose(0, 2, 1, 3).reshape(KT // 2, 128, 2 * G).astype(F8))
    bbc = np.ascontiguousarray(
        np.broadcast_to(np.asarray(b, np.float32)[None, :], (128, G)).astype(BF16))
    ident = np.ascontiguousarray(np.eye(128, dtype=np.float32).astype(BF16))

    ht0_blocks = [None] * NCORES
    c0_blocks = [None] * NCORES
    if not zero_init:
        h1 = np.asarray(h1, np.float32); h2 = np.asarray(h2, np.float32)
        c1 = np.asarray(c1, np.float32); c2 = np.asarray(c2, np.float32)
        hin = np.where(sch["chain_player"][:, None] > 0, h1[sch["chain_b"]],
                       h2[sch["chain_b"]])
        cin_ = np.where(sch["chain_player"][:, None] > 0, c1[sch["chain_b"]],
                        c2[sch["chain_b"]])
        live = sch["chain_seg"] == 0
        hin = np.where(live[:, None], hin, 0.0)
        cin_ = np.where(live[:, None], cin_, 0.0)
        M0 = int(M[0])
        for c in range(NCORES):
            selc = sch["core_of_chain"] == c
            rows = sch["core_rank"][selc]
            Hp = np.zeros((M0 * 128, H), np.float32)
            Cp = np.zeros((M0 * 128, H), np.float32)
            Hp[rows] = hin[selc]
            Cp[rows] = cin_[selc]
            # transposed layout for lhsT: [k, p, col] = Hp[col, k*128+p]
            ht0 = Hp.reshape(M0 * 128, KT, 128).transpose(1, 2, 0).astype(F8)
            ht0_blocks[c] = np.ascontiguousarray(ht0)
            c0_blocks[c] = np.ascontiguousarray(Cp)

    return dict(
        sch=sch, zero_init=zero_init, Mzx=Mzx, zx_row0=zx_row0, zx_rows=zx_rows,
        zx_start_tile=zx_start_tile, xt_blocks=xt_blocks, Wi_l=Wi_l, Wh_l=Wh_l,
        bbc=bbc, ident=ident, ht0_blocks=ht0_blocks, c0_blocks=c0_blocks,
        core_pos=core_pos, padded_row=padded_row, T=T, B=B,
    )


# ---------------------------------------------------------------------------
# Device program
# ---------------------------------------------------------------------------

def _build_program(D, U, M, V, P, Mzx, zx_row0, zx_rows, zx_start_tile,
                   zero_init, no_tail=False):
    nc = bacc.Bacc("TRN2", target_bir_lowering=False, debug=False)

    M0 = int(M[0])
    Ptot = int(P[D])
    d_start = 1 if zero_init else 0
    need_zx = D > d_start
    nzx_rows = zx_rows
    nwave1 = int(M[d_start]) if D > d_start else 0  # m-tiles of first wave

    xt_d = nc.dram_tensor("xt", [Mzx, 128, CIN], DT.bfloat16, kind="ExternalInput")
    wi_d = nc.dram_tensor("wi", [2, KT, 128, 2048], DT.bfloat16,
                          kind="ExternalInput")
    wh_d = nc.dram_tensor("wh", [KT // 2, 128, 2 * G], DT.float8e4,
                          kind="ExternalInput")
    bbc_d = nc.dram_tensor("bbc", [128, G], DT.bfloat16, kind="ExternalInput")
    id_d = nc.dram_tensor("ident", [128, 128], DT.bfloat16, kind="ExternalInput")
    y_d = nc.dram_tensor("y", [Ptot, H], DT.bfloat16, kind="ExternalOutput")
    if need_zx:
        zx_d = nc.dram_tensor("zx", [max(nzx_rows, 128), G], DT.bfloat16,
                              kind="Internal")
        zx_ap = zx_d.ap()
    hs_d = nc.dram_tensor("hstate", [Ptot, H], DT.bfloat16, kind="Internal")
    cs_d = nc.dram_tensor("cstate", [Ptot, H], DT.bfloat16, kind="Internal")
    if not zero_init:
        ht0_d = nc.dram_tensor("ht0", [KT, 128, M0 * 128], DT.float8e4,
                               kind="ExternalInput")
        c0_d = nc.dram_tensor("c0", [M0 * 128, H], DT.float32, kind="ExternalInput")

    xt_ap = xt_d.ap(); y_ap = y_d.ap(); hs_ap = hs_d.ap(); cs_ap = cs_d.ap()

    # Waves that keep state SBUF-resident: single m-tile, and the previous
    # wave small enough that its first m-tile's state tiles are still alive
    # (state-pool bufs=3 below).  M is monotone nonincreasing, so once a wave
    # qualifies every later wave does too.
    sbuf_in = [False] * D   # wave d takes h/c from prev wave's SBUF tiles
    for d in range(d_start + 1, D):
        if int(M[d]) == 1 and int(M[d - 1]) <= 3:
            sbuf_in[d] = True
    import os
    if no_tail or os.environ.get("KERNEL_NO_SBUF_TAIL"):
        sbuf_in = [False] * D

    # Wave d_start's h^T built during phase A from its per-tile hs stores
    w1_pre = zero_init and D > 1 and not no_tail \
        and not os.environ.get("KERNEL_NO_W1_PRE")

    with tile.TileContext(nc) as tc:
        from contextlib import ExitStack
        with ExitStack() as es:
            const = es.enter_context(tc.tile_pool(name="const", bufs=1))
            work = es.enter_context(tc.tile_pool(name="work", bufs=2))
            psum = es.enter_context(tc.tile_pool(name="psum", bufs=1, space="PSUM"))
            whPa = es.enter_context(tc.tile_pool(name="whPa", bufs=1))

            ident_sb = const.tile([128, 128], DT.bfloat16, tag="ident")
            # Wh lives entirely in SBUF as fp8 DoubleRow pairs (32KB/part),
            # prefetched during phase A: no phase-boundary weight loads.
            wh8_sb = whPa.tile([128, (KT // 2) * 2 * G], DT.float8e4,
                               tag="wh8", name="wh8_sb")
            if w1_pre:
                hT1P = es.enter_context(tc.tile_pool(name="hT1P", bufs=1))
                hT_w1 = hT1P.tile([128, KT * nwave1 * 128], DT.bfloat16,
                                  tag="hT1", name="hT_w1")
                hT_w1_f8 = hT1P.tile([128, KT * nwave1 * 128], DT.float8e4,
                                     tag="hT1f8", name="hT_w1_f8")

            def gate_math(z_src, c_src, out_row, store_h, store_c):
                """LSTM gate math for one 128-row m-tile.

                z_src: [128, G] bf16 SBUF tile holding z
                c_src: [128, H] fp32 ap with previous c, or None (c == 0;
                       the f gate is then unused and its sigmoid skipped).
                Returns (ncv, nhb) tiles (fp32 cell state, bf16 hidden).
                """
                gi = work.tile([128, H], DT.bfloat16, tag="gi", name="gi", bufs=1)
                gg = work.tile([128, H], DT.bfloat16, tag="gg", name="gg", bufs=1)
                go = work.tile([128, H], DT.bfloat16, tag="go", name="go", bufs=1)
                nc.scalar.activation(gi[:], z_src[:, 0 * H:1 * H], AF.Sigmoid)
                nc.scalar.activation(gg[:], z_src[:, 2 * H:3 * H], AF.Tanh)
                ncv = work.tile([128, H], DT.float32, tag="ncv", name="ncv",
                                bufs=3)
                if c_src is not None:
                    gf = work.tile([128, H], DT.bfloat16, tag="gf", name="gf",
                                   bufs=1)
                    nc.scalar.activation(gf[:], z_src[:, 1 * H:2 * H], AF.Sigmoid)
                    m1 = work.tile([128, H], DT.bfloat16, tag="f32a", name="m1",
                                   bufs=1)
                    nc.vector.tensor_mul(m1[:], gi[:], gg[:])
                    t1 = work.tile([128, H], DT.bfloat16, tag="f32b", name="t1",
                                   bufs=1)
                    nc.vector.tensor_mul(t1[:], gf[:], c_src[:])
                    nc.vector.tensor_add(ncv[:], t1[:], m1[:])
                else:
                    nc.vector.tensor_mul(ncv[:], gi[:], gg[:])
                nc.scalar.activation(go[:], z_src[:, 3 * H:4 * H], AF.Sigmoid)
                tnc = work.tile([128, H], DT.bfloat16, tag="f32b", name="tnc",
                                bufs=1)
                nc.scalar.activation(tnc[:], ncv[:], AF.Tanh)
                # nh produced directly in bf16 (y is stored bf16; host upcasts)
                nhb = work.tile([128, H], DT.bfloat16, tag="nhb", name="nhb",
                                bufs=3)
                nc.vector.tensor_mul(nhb[:], go[:], tnc[:])

                nc.gpsimd.dma_start(out=y_ap[out_row:out_row + 128, :], in_=nhb[:])
                if store_h:
                    nc.sync.dma_start(out=hs_ap[out_row:out_row + 128, :],
                                      in_=nhb[:])
                if store_c:
                    ncb = work.tile([128, H], DT.bfloat16, tag="ncb",
                                    name="ncb", bufs=2)
                    nc.vector.tensor_copy(ncb[:], ncv[:])
                    nc.sync.dma_start(out=cs_ap[out_row:out_row + 128, :],
                                      in_=ncb[:])
                return ncv, nhb

            def mm_half(lhsT_of_k, rhs_of_k, half, k_order=None):
                """Half an m-tile of z accumulation (512 of 1024 cols/gate).

                PSUM tags alternate by half (8 tags x 1 buf = all 8 banks), so
                one half's matmuls overlap the other half's drain, both within
                an m-tile and across m-tiles/waves.
                """
                pt = []
                for g in range(4):
                    pt.append(psum.tile([128, 512], DT.float32,
                                        tag=f"p{half}{g}", name=f"p{half}{g}",
                                        bufs=1))
                ks = list(k_order) if k_order is not None else list(range(KT))
                for i, k in enumerate(ks):
                    lhsT = lhsT_of_k(k)
                    for g in range(4):
                        col0 = g * H + half * 512
                        nc.tensor.matmul(
                            pt[g][:], lhsT=lhsT, rhs=rhs_of_k(k, col0),
                            start=(i == 0), stop=(i == len(ks) - 1),
                            skip_group_check=True)
                return pt

            # ---------------- phase A: zx = x@Wi + bias ----------------
            with tc.tile_pool(name="wiP", bufs=1) as wiP, \
                 tc.tile_pool(name="xtP", bufs=2) as xtP:
                # wi_sb layout matches the DRAM demand order:
                # [:, (half*KT + k)*2048 + g*512 + c]
                wi_sb = wiP.tile([128, KT * G], DT.bfloat16, tag="wi", name="wi_sb")
                # sync queue (HW DGE) carries the PE-critical startup loads in
                # exact consumption order; everything else goes via gpsimd
                xt0_sb = xtP.tile([128, CIN], DT.bfloat16, tag="xt", name="xt_sb")
                nc.sync.dma_start(out=xt0_sb[:], in_=xt_ap[0])
                # wi arrives in exact consumption order: the first half-0
                # k-slices ride the sync queue (parallel channel to gpsimd),
                # the rest stream per-slice via gpsimd
                for g in range(4):
                    nc.sync.dma_start(
                        out=wi_sb[:, g * 512:(g + 1) * 512],
                        in_=wi_d.ap()[0][0][:, g * 512:(g + 1) * 512])
                for k in range(1, 3):
                    off = k * 2048
                    nc.sync.dma_start(out=wi_sb[:, off:off + 2048],
                                      in_=wi_d.ap()[0][k])
                for k in range(3, KT):
                    off = k * 2048
                    nc.gpsimd.dma_start(out=wi_sb[:, off:off + 2048],
                                        in_=wi_d.ap()[0][k])
                h1off = KT * 2048
                nc.gpsimd.dma_start(
                    out=wi_sb[:, h1off:h1off + 4 * 2048].rearrange(
                        "p (k c) -> p k c", k=4),
                    in_=wi_d.ap()[1][0:4].rearrange("k p c -> p k c"))
                for k in range(4, KT):
                    off = h1off + k * 2048
                    nc.gpsimd.dma_start(out=wi_sb[:, off:off + 2048],
                                        in_=wi_d.ap()[1][k])
                bbc_sb = wiP.tile([128, G], DT.bfloat16, tag="bbc", name="bbc_sb")
                nc.sync.dma_start(out=bbc_sb[:], in_=bbc_d.ap()[:])
                nc.sync.dma_start(out=ident_sb[:], in_=id_d.ap()[:])

                def wi_rhs(k, col0):
                    g, within = divmod(col0, H)
                    off = ((within // 512) * KT + k) * 2048 + g * 512
                    return wi_sb[:, off:off + 512]

                # Tile emission order: wave-1-feeding prefix first (gate math
                # + h/c stores + per-tile h^T transpose chunks), then the
                # wave-zx tiles, then the remaining depth-0 tiles.  All wave
                # inputs are thus ready long before phase B starts.
                if zero_init and D > 1:
                    g1 = list(range(min(nwave1, Mzx)))
                    g2 = [t for t in range(zx_start_tile, Mzx) if t not in g1]
                    g3 = [t for t in range(nwave1, min(zx_start_tile, Mzx))]
                    order = g1 + g2 + g3
                else:
                    order = list(range(Mzx))
                wh_pref_at = min(nwave1 + 2, max(0, len(order) - 1))

                for idx, mt in enumerate(order):
                    if (idx == 2 and need_zx
                            and nzx_rows > Mzx * 128 - zx_row0):
                        # zero the zx rows past the packed region that deep
                        # waves read as padding: the identity-inject matmul
                        # would otherwise spread NaNs from uninitialized
                        # DRAM into every output row
                        gr0 = Mzx * 128 - zx_row0
                        zpad = work.tile([128, G], DT.bfloat16, tag="zws",
                                         name="zpad", bufs=3)
                        nc.vector.memset(zpad[:], 0)
                        nc.gpsimd.dma_start(out=zx_ap[gr0:nzx_rows, :],
                                            in_=zpad[0:nzx_rows - gr0, :])
                    if idx == wh_pref_at and (need_zx or D > 1):
                        for kp in range(KT // 2):
                            nc.gpsimd.dma_start(
                                out=wh8_sb[:, kp * 2 * G:(kp + 1) * 2 * G],
                                in_=wh_d.ap()[kp])
                    if mt == 0:
                        xt_sb = xt0_sb
                    else:
                        xt_sb = xtP.tile([128, CIN], DT.bfloat16, tag="xt",
                                         name="xt_sb")
                        eng = nc.sync if idx <= 2 else nc.gpsimd
                        eng.dma_start(out=xt_sb[:], in_=xt_ap[mt])

                    zxt = work.tile([128, G], DT.bfloat16, tag="zws", name="zxt",
                                    bufs=3)
                    for half in range(2):
                        pt = mm_half(
                            lambda k: xt_sb[:, k * 128:(k + 1) * 128],
                            wi_rhs, half)
                        # psum -> SBUF with bias add (bf16)
                        for g in range(4):
                            col0 = g * H + half * 512
                            nc.vector.tensor_add(zxt[:, col0:col0 + 512],
                                                 pt[g][:],
                                                 bbc_sb[:, col0:col0 + 512])

                    if need_zx and mt >= zx_start_tile:
                        r = mt * 128 - zx_row0
                        nc.gpsimd.dma_start(out=zx_ap[r:r + 128, :], in_=zxt[:])
                    if zero_init and mt < M0:
                        nwv = nwave1 if D > 1 else 0
                        st = D > 1 and mt < nwv
                        gate_math(zxt, None, mt * 128, store_h=st, store_c=st)
                        if w1_pre and mt < nwv:
                            # build wave-1's h^T chunk from the rows just
                            # stored; overlaps the remaining phase-A tiles
                            nc.sync.dma_start_transpose(
                                out=hT_w1.rearrange(
                                    "p (j m) -> p j m",
                                    m=nwave1 * 128)[:, :,
                                                    mt * 128:(mt + 1) * 128],
                                in_=hs_ap[mt * 128:(mt + 1) * 128, :])
                            if mt == nwv - 1:
                                nc.vector.tensor_copy(hT_w1_f8[:], hT_w1[:])

            # ---------------- phase B: waves ----------------
            if D > d_start:
                with tc.tile_pool(name="hTP", bufs=2) as hTP, \
                     tc.tile_pool(name="zxP", bufs=2) as zxP, \
                     tc.tile_pool(name="cP", bufs=2) as cP:
                    # pair kp covers k-slices (2kp, 2kp+1); pairs 2..3 are
                    # h-features 512..1023 (produced first by the half-1
                    # drain), so iterate them first
                    KP_ORDER = [2, 3, 0, 1]
                    NKP = KT // 2
                    DR = mybir.MatmulPerfMode.DoubleRow

                    def wh_rhs(kp, col0):
                        return wh8_sb[:, kp * 2 * G:(kp + 1) * 2 * G] \
                            .rearrange("p (i n) -> p i n", i=2)[
                                :, :, col0:col0 + 512]

                    def wave_mm(lhsT_of_kp, half, inject=None, at_i=None):
                        """One half's PSUM groups: optional bf16 zx inject
                        opener + 4 fp8 DoubleRow matmuls (K=256 each).
                        at_i(i) is called before pair i's matmuls (used to
                        slot the deferred h^T transposes into the PE queue
                        at the right point)."""
                        pt = []
                        for g in range(4):
                            pt.append(psum.tile([128, 512], DT.float32,
                                                tag=f"p{half}{g}",
                                                name=f"p{half}{g}", bufs=1))
                        if inject is not None:
                            for g in range(4):
                                col0 = g * H + half * 512
                                nc.tensor.matmul(
                                    pt[g][:], lhsT=ident_sb[:],
                                    rhs=inject[:, col0:col0 + 512],
                                    start=True, stop=False,
                                    skip_group_check=True)
                        for i, kp in enumerate(KP_ORDER):
                            if at_i is not None:
                                at_i(i)
                            for g in range(4):
                                col0 = g * H + half * 512
                                nc.tensor.matmul(
                                    pt[g][:], lhsT=lhsT_of_kp(kp),
                                    rhs=wh_rhs(kp, col0),
                                    start=(i == 0 and inject is None),
                                    stop=(i == NKP - 1),
                                    perf_mode=DR, skip_group_check=True)
                        return pt

                    if w1_pre:
                        dram_ds = [d for d in range(max(d_start, 1) + 1, D)
                                   if not sbuf_in[d]]
                        hT_cols_max = (KT * max(int(M[d]) for d in dram_ds) * 128
                                       if dram_ds else KT * 128)
                    else:
                        hT_cols_max = KT * int(M[max(d_start, 1)]) * 128 \
                            if D > max(d_start, 1) else KT * 128

                    def pair_view(hT8, ncols_, mt):
                        """[128, 2, 128] DoubleRow lhsT view of block pair
                        kp at m-tile mt of an fp8 h^T of ncols_ per block."""
                        v = hT8[:, 0:KT * ncols_].rearrange(
                            "p (j m) -> p j m", m=ncols_)
                        return lambda kp: v[:, 2 * kp:2 * kp + 2,
                                            mt * 128:mt * 128 + 128]

                    def emit_T(blocks, src_q, dst_hT):
                        """PE-transpose 128-col blocks of nh quarter tiles
                        into dst h^T (fp8 cast on the copy).  src_q maps
                        block index kk -> its [128, 256] quarter tile."""
                        for j, kk in enumerate(blocks):
                            htag = 1 if kk >= 4 else 0
                            ptT = psum.tile([128, 128], DT.bfloat16,
                                            tag=f"p{htag}{j}", name=f"ptT{kk}",
                                            bufs=1)
                            c0 = (kk % 2) * 128
                            nc.tensor.matmul(
                                ptT[:], lhsT=src_q[kk // 2][:, c0:c0 + 128],
                                rhs=ident_sb[:], is_transpose=True,
                                start=True, stop=True, skip_group_check=True)
                            nc.vector.tensor_copy(
                                dst_hT[:, kk * 128:(kk + 1) * 128], ptT[:])

                    prev_ncv = None
                    prev_nhb = None
                    pending_hT = None   # hT pre-built by the previous tail wave
                    pending_T0 = None   # deferred emission of hT blocks 0..3
                    pre_hT = None       # next DRAM wave's hT, built early
                    tail_zx = {}        # tail zx tiles prefetched a wave ahead

                    for d in range(d_start, D):
                        Md = int(M[d])
                        ncols = Md * 128

                        if sbuf_in[d]:
                            # ---- SBUF-resident tail wave (one m-tile) ----
                            if d in tail_zx:
                                zx_sb = tail_zx.pop(d)
                            else:
                                zx_sb = zxP.tile([128, G], DT.bfloat16,
                                                 tag="zx", name="zx_sb",
                                                 bufs=3)
                                r = int(V[d]) - zx_row0
                                nc.sync.dma_start(out=zx_sb[:],
                                                  in_=zx_ap[r:r + 128, :])
                            if d + 1 < D and sbuf_in[d + 1]:
                                nxt_zx = zxP.tile([128, G], DT.bfloat16,
                                                  tag="zx", name="zx_nxt",
                                                  bufs=3)
                                rn = int(V[d + 1]) - zx_row0
                                nc.sync.dma_start(out=nxt_zx[:],
                                                  in_=zx_ap[rn:rn + 128, :])
                                tail_zx[d + 1] = nxt_zx

                            if pending_hT is not None:
                                hT = pending_hT
                            else:
                                hTstage = hTP.tile([128, KT * 128],
                                                   DT.bfloat16, tag="hTsS",
                                                   name="hTstage")
                                nc.sync.dma_start_transpose(
                                    out=hTstage.rearrange("p (j c) -> p j c",
                                                          c=128),
                                    in_=prev_nhb[:])
                                hT = hTP.tile([128, KT * 128], DT.float8e4,
                                              tag="hTs", name="hTs")
                                nc.scalar.activation(hT[:], hTstage[:],
                                                     AF.Copy)

                            build_next = d + 1 < D
                            if build_next:
                                hT_next = hTP.tile([128, KT * 128],
                                                   DT.float8e4, tag="hTs",
                                                   name="hTn")
                            ncv = work.tile([128, H], DT.float32, tag="ncv",
                                            name="ncv", bufs=3)
                            # nh in 256-col quarters: each feeds its h^T
                            # transposes the moment its subchain completes
                            nhq = [work.tile([128, 256], DT.bfloat16,
                                             tag=f"nhq{q}", name=f"nhq{q}",
                                             bufs=2) for q in range(4)]
                            lhsT_kp = pair_view(hT, 128, 0)

                            def tail_at_i(i):
                                nonlocal pending_T0
                                if i == 2 and pending_T0 is not None:
                                    pending_T0()
                                    pending_T0 = None

                            def tail_drain(half, pt):
                                for s in range(2):
                                    q = half * 2 + s
                                    sub = slice(s * 256, s * 256 + 256)
                                    blk = slice(half * 512 + s * 256,
                                                half * 512 + s * 256 + 256)
                                    gi = work.tile([128, 256], DT.bfloat16,
                                                   tag="gi", name="gi", bufs=1)
                                    gf = work.tile([128, 256], DT.bfloat16,
                                                   tag="gf", name="gf", bufs=1)
                                    gg = work.tile([128, 256], DT.bfloat16,
                                                   tag="gg", name="gg", bufs=1)
                                    go = work.tile([128, 256], DT.bfloat16,
                                                   tag="go", name="go", bufs=1)
                                    nc.scalar.activation(gf[:], pt[1][:, sub],
                                                         AF.Sigmoid)
                                    nc.scalar.activation(gi[:], pt[0][:, sub],
                                                         AF.Sigmoid)
                                    nc.scalar.activation(gg[:], pt[2][:, sub],
                                                         AF.Tanh)
                                    nc.scalar.activation(go[:], pt[3][:, sub],
                                                         AF.Sigmoid)
                                    t1b = work.tile([128, 256], DT.float32,
                                                    tag="t1b", name="t1b",
                                                    bufs=1)
                                    nc.vector.tensor_mul(t1b[:], gf[:],
                                                         prev_ncv[:, blk])
                                    m1b = work.tile([128, 256], DT.float32,
                                                    tag="m1b", name="m1b",
                                                    bufs=1)
                                    nc.vector.tensor_mul(m1b[:], gi[:], gg[:])
                                    nc.vector.tensor_add(ncv[:, blk], t1b[:],
                                                         m1b[:])
                                    tncb = work.tile([128, 256], DT.float32,
                                                     tag="tncb", name="tncb",
                                                     bufs=1)
                                    nc.scalar.activation(tncb[:], ncv[:, blk],
                                                         AF.Tanh)
                                    nc.vector.tensor_mul(nhq[q][:], go[:],
                                                         tncb[:])

                            # half 1 first: the next wave's matmuls consume
                            # feature pairs 2..3 first, built from this half.
                            # emit_T(4..7) sits between half 0's matmuls and
                            # its drain so the scalar queue runs the h^T
                            # copies before half 0's ACTs.
                            qmap = {2: nhq[2], 3: nhq[3], 0: nhq[0],
                                    1: nhq[1]}
                            pt1 = wave_mm(lhsT_kp, 1, inject=zx_sb,
                                          at_i=tail_at_i)
                            tail_drain(1, pt1)
                            nc.sync.dma_start(
                                out=y_ap[int(P[d]):int(P[d]) + 128, 512:768],
                                in_=nhq[2][:])
                            nc.sync.dma_start(
                                out=y_ap[int(P[d]):int(P[d]) + 128, 768:1024],
                                in_=nhq[3][:])
                            pt0 = wave_mm(lhsT_kp, 0, inject=zx_sb)
                            if build_next:
                                emit_T((4, 5, 6, 7), qmap, hT_next)
                            tail_drain(0, pt0)
                            nc.sync.dma_start(
                                out=y_ap[int(P[d]):int(P[d]) + 128, 0:256],
                                in_=nhq[0][:])
                            nc.sync.dma_start(
                                out=y_ap[int(P[d]):int(P[d]) + 128, 256:512],
                                in_=nhq[1][:])
                            if build_next:
                                # blocks 0..3 deferred into the next wave's
                                # pair loop so the PE never waits on the drain
                                pending_T0 = (lambda q_=qmap, hTn=hT_next:
                                              emit_T((0, 1, 2, 3), q_, hTn))

                            prev_ncv = ncv
                            pending_hT = hT_next if build_next else None
                            continue

                        # ---- DRAM-path wave ----
                        pending_hT = None
                        pending_T0 = None
                        if d == 0:
                            hT = None  # allocated per m-tile below
                        elif d == d_start and w1_pre:
                            hT = hT_w1_f8
                        elif pre_hT is not None:
                            hT = pre_hT  # transpose already ran mid-prev-wave
                        else:
                            hTstage = hTP.tile([128, hT_cols_max],
                                               DT.bfloat16, tag="hT",
                                               name="hTstage")
                            prev = int(P[d - 1])
                            nc.sync.dma_start_transpose(
                                out=hTstage[:, 0:KT * ncols].rearrange(
                                    "p (j m) -> p j m", m=ncols),
                                in_=hs_ap[prev:prev + ncols, :])
                            hT = hTP.tile([128, hT_cols_max], DT.float8e4,
                                          tag="hT8", name="hT8")
                            nc.vector.tensor_copy(hT[:, 0:KT * ncols],
                                                  hTstage[:, 0:KT * ncols])
                        pre_hT = None
                        prev_P = int(P[d - 1]) if d > 0 else 0
                        deferred = []
                        nxt_sbuf = (d + 1 < D) and sbuf_in[d + 1]
                        nwave = int(M[d + 1]) * 128 if d + 1 < D else 0

                        def drain_deferred(d=d, nxt_sbuf=nxt_sbuf,
                                           nwave=nwave):
                            nonlocal prev_ncv, prev_nhb, pre_hT
                            z_p, c_p, mt_p = deferred.pop(0)
                            ncv, nhb = gate_math(
                                z_p, c_p, int(P[d]) + mt_p * 128,
                                store_h=(d + 1 < D and not nxt_sbuf
                                         and mt_p * 128 < nwave),
                                store_c=(d + 1 < D and not nxt_sbuf
                                         and mt_p * 128 < nwave))
                            if mt_p == 0:
                                prev_ncv, prev_nhb = ncv, nhb
                            # once the next DRAM wave's source rows are all
                            # stored, run its h^T transpose (overlaps the
                            # remaining m-tiles)
                            if (d + 1 < D and not nxt_sbuf
                                    and mt_p == int(M[d + 1]) - 1):
                                ncols_n = int(M[d + 1]) * 128
                                preS = hTP.tile([128, hT_cols_max],
                                                DT.bfloat16, tag="hT",
                                                name="hTpreS")
                                nc.sync.dma_start_transpose(
                                    out=preS[:, 0:KT * ncols_n].rearrange(
                                        "p (j m) -> p j m", m=ncols_n),
                                    in_=hs_ap[int(P[d]):int(P[d]) + ncols_n,
                                              :])
                                pre_hT = hTP.tile([128, hT_cols_max],
                                                  DT.float8e4, tag="hT8",
                                                  name="hTpre")
                                nc.scalar.activation(
                                    pre_hT[:, 0:KT * ncols_n],
                                    preS[:, 0:KT * ncols_n], AF.Copy)

                        for mt in range(Md):
                            zx_sb = zxP.tile([128, G], DT.bfloat16, tag="zx",
                                             name="zx_sb", bufs=3)
                            r = int(V[d]) + mt * 128 - zx_row0
                            nc.gpsimd.dma_start(out=zx_sb[:],
                                                in_=zx_ap[r:r + 128, :])

                            if d == 0:
                                c_src = cP.tile([128, H], DT.float32, tag="c",
                                                name="c_sb")
                                nc.gpsimd.dma_start(
                                    out=c_src[:],
                                    in_=c0_d.ap()[mt * 128:mt * 128 + 128, :])
                            else:
                                c_src = cP.tile([128, H], DT.bfloat16,
                                                tag="c", name="c_sb")
                                nc.gpsimd.dma_start(
                                    out=c_src[:],
                                    in_=cs_ap[prev_P + mt * 128:
                                              prev_P + mt * 128 + 128, :])

                            if d == 0:
                                hT = hTP.tile([128, KT * 128], DT.float8e4,
                                              tag="hT0", name="hT0")
                                for k in range(KT):
                                    nc.gpsimd.dma_start(
                                        out=hT[:, k * 128:(k + 1) * 128],
                                        in_=ht0_d.ap()[k][:, mt * 128:
                                                          mt * 128 + 128])
                                lhsT_kp = pair_view(hT, 128, 0)
                            elif d == d_start and w1_pre:
                                lhsT_kp = pair_view(hT, nwave1 * 128, mt)
                            else:
                                lhsT_kp = pair_view(hT, ncols, mt)

                            z_sb = work.tile([128, G], DT.bfloat16, tag="zws",
                                             name="z_sb", bufs=3)
                            for half in range(2):
                                pt = wave_mm(lhsT_kp, half)
                                for g in range(4):
                                    col0 = g * H + half * 512
                                    nc.vector.tensor_add(
                                        z_sb[:, col0:col0 + 512], pt[g][:],
                                        zx_sb[:, col0:col0 + 512])

                            # defer the gate math by one m-tile: the z-adds
                            # (which free PSUM slots) stay at the head of the
                            # DVE queue, so the next tile's/wave's matmuls
                            # are never blocked behind trailing gate work
                            deferred.append((z_sb, c_src, mt))
                            # mt0 of the wave feeding the first SBUF tail
                            # wave must not be deferred: the tail's h^T xbar
                            # and prev_ncv consume it as early as possible
                            if len(deferred) > 1 or (mt == 0 and nxt_sbuf):
                                drain_deferred()

                        while deferred:
                            drain_deferred()

    nc.compile()
    return nc


# ---------------------------------------------------------------------------
# Entry point
# ---------------------------------------------------------------------------

_PROGRAM_CACHE = {}


def _run(inputs, trace=False):
    prep = _prep_inputs(**inputs)
    sch = prep["sch"]
    D, U, M, V, P = sch["D"], sch["U"], sch["M"], sch["V"], sch["P"]

    in_maps = []
    for c in range(NCORES):
        m = {
            "xt": prep["xt_blocks"][c],
            "wi": prep["Wi_l"],
            "wh": prep["Wh_l"],
            "bbc": prep["bbc"],
            "ident": prep["ident"],
        }
        if not prep["zero_init"]:
            m["ht0"] = prep["ht0_blocks"][c]
            m["c0"] = prep["c0_blocks"][c]
        in_maps.append(m)

    # Retry ladder: rare transient device errors have been observed on the
    # shared terminal; retry twice, then once more with the conservative
    # (no SBUF-resident tail waves) program variant.
    import time as _time
    res = None
    last_err = None
    for attempt, no_tail in enumerate([False, False, True]):
        key = (D, tuple(M.tolist()), tuple(U.tolist()), prep["Mzx"],
               prep["zx_row0"], prep["zx_rows"], prep["zero_init"], no_tail)
        try:
            if key not in _PROGRAM_CACHE:
                _PROGRAM_CACHE[key] = _build_program(
                    D, U, M, V, P, prep["Mzx"], prep["zx_row0"],
                    prep["zx_rows"], prep["zx_start_tile"], prep["zero_init"],
                    no_tail=no_tail)
            nc = _PROGRAM_CACHE[key]
            res = run_bass_kernel_spmd(nc, in_maps,
                                       core_ids=list(range(NCORES)),
                                       trace=trace)
            break
        except Exception as e:  # noqa: BLE001 - retry on device hiccups
            last_err = e
            sys.stderr.write(f"kernel attempt {attempt} failed: {e!r}\n")
            trace = False  # profiling hook may be wedged; drop it on retry
            _time.sleep(2.0)
    if res is None:
        raise last_err

    T, B = prep["T"], prep["B"]
    y_full = np.empty((T * B, H), np.float32)
    core_pos = prep["core_pos"]; padded_row = prep["padded_row"]
    for c in range(NCORES):
        sel = core_pos == c
        y_full[sel] = res.results[c]["y"][padded_row[sel]].astype(np.float32)
    return y_full, res


def kernel(**inputs) -> np.ndarray:
    y, _ = _run(inputs, trace=False)
    return y


# revision 48
# speedup vs baseline: 1.0128x; 1.0128x over previous
"""Trainium2 Bass kernel for the 2-player masked LSTM scan.

Reference semantics (T=128 steps, B=256 batch, C=1024 in, H=1024 hidden):
  per step t, batch b: the active player's (c,h) (selected by main[t,b]) runs
  one LSTM cell z = x@Wi + h@Wh + b with fused i,f,g,o gates; the result is
  written back only to the active player's state, and both players' states are
  zeroed where done[t,b].

Key algorithmic idea: done/main are *inputs*, so the true dependency structure
is known on the host.  Each (b, segment, player) triple forms an independent
"chain" of positions; a position at depth d in its chain depends only on the
position at depth d-1.  With done ~ Bernoulli(0.5) per step, chains are short
(max depth ~9 for the target inputs), so the sequential scan of 128 steps
becomes ~9 dense "waves", each a full-batch matmul with no masking at all.

Host side: sort chains by length (desc), round-robin across the 8 cores, lay
positions out wave-major.  Because chain order is sorted by length, the chains
alive at depth d are exactly a prefix of those alive at depth d-1 - so wave d
reads a contiguous prefix of wave d-1's outputs: no gather needed on device.

Device: phase A computes zx = x@Wi (+ bias via DVE) for all positions, fusing
the full gate math for depth-0 positions (input state is zero); phase B runs
one matmul z = zx + h@Wh per wave plus the LSTM gate math.  bf16 matmuls with
fp32 PSUM accumulation; the carried cell state c stays fp32.

Scheduling: phase A emits the wave-1-feeding prefix tiles first, then the
wave-zx tiles, then the remaining depth-0 tiles, so wave 1's h-transpose and
zx loads complete long before the waves start (no phase-boundary PE stall).
Tail waves (one m-tile) keep all state in SBUF; h^T for the next wave is
built with PE transpose instructions (no xbar DMA on the critical path), the
zx injection runs as the PSUM-group opener so it executes during the previous
wave's drain, and per-half alternating PSUM tags let consecutive waves'
matmuls overlap each other's drains.
"""

import sys

sys.path.insert(0, "/opt/trn_rl_repo")

import numpy as np
import ml_dtypes

import concourse.bass as bass
import concourse.tile as tile
from concourse import bacc, mybir
from concourse.bass_utils import run_bass_kernel_spmd

BF16 = ml_dtypes.bfloat16
F8 = ml_dtypes.float8_e4m3
AF = mybir.ActivationFunctionType
DT = mybir.dt

NCORES = 8
H = 1024
CIN = 1024
G = 4 * H  # 4096 fused gate width
KT = CIN // 128  # 8 k-tiles for both Wi and Wh contractions


# ---------------------------------------------------------------------------
# Host-side schedule construction
# ---------------------------------------------------------------------------

def _build_schedule(done, main, T, B):
    """Chain decomposition of the (t, b) grid.

    Returns per-position (core, depth, rank) and the uniform padded wave
    geometry shared by all cores (SPMD requires identical programs).
    """
    done2 = done.reshape(T, B).astype(bool)
    main2 = main.reshape(T, B).astype(bool)

    seg = np.zeros((T, B), np.int64)
    if T > 1:
        seg[1:] = np.cumsum(done2[:-1], axis=0)
    player = main2.astype(np.int64)
    key = (np.arange(B)[None, :] * (T + 1) + seg) * 2 + player  # [T, B]
    flat_key = key.reshape(-1)  # position p = t*B + b
    order = np.argsort(flat_key, kind="stable")  # chain-major, t-ascending
    sorted_keys = flat_key[order]
    uk, first_idx, inv = np.unique(sorted_keys, return_index=True, return_inverse=True)
    chain_len = np.diff(np.append(first_idx, len(sorted_keys)))
    npos = T * B

    depth = np.empty(npos, np.int64)
    depth[order] = np.arange(npos) - first_idx[inv]
    chain_id = np.empty(npos, np.int64)
    chain_id[order] = inv

    n_chains = len(uk)
    chain_b = uk // (2 * (T + 1))
    chain_seg = (uk // 2) % (T + 1)
    chain_player = uk % 2

    chain_order = np.argsort(-chain_len, kind="stable")
    rank_of_chain = np.empty(n_chains, np.int64)
    rank_of_chain[chain_order] = np.arange(n_chains)
    core_of_chain = (rank_of_chain % NCORES).astype(np.int64)
    core_rank = rank_of_chain // NCORES

    D = int(chain_len.max())
    lens_sorted = np.sort(chain_len)
    N_d = np.array([n_chains - np.searchsorted(lens_sorted, d, side="right")
                    for d in range(D)], np.int64)
    U = np.ceil(N_d / NCORES).astype(np.int64)      # uniform per-core wave rows
    M = np.ceil(U / 128).astype(np.int64)           # padded wave m-tiles
    V = np.concatenate([[0], np.cumsum(U)])          # packed row offsets
    P = np.concatenate([[0], np.cumsum(M * 128)])    # padded row offsets

    return dict(
        depth=depth, chain_id=chain_id, core_of_chain=core_of_chain,
        core_rank=core_rank, chain_b=chain_b, chain_seg=chain_seg,
        chain_player=chain_player, D=D, U=U, M=M, V=V, P=P,
    )


def _prep_inputs(x, c1, h1, c2, h2, Wi, Wh, b, done, main):
    """Build per-core device input arrays + output scatter indices."""
    B = c1.shape[0]
    T = x.shape[0] // B
    sch = _build_schedule(np.asarray(done), np.asarray(main), T, B)
    D, U, M, V, P = sch["D"], sch["U"], sch["M"], sch["V"], sch["P"]

    zero_init = not (np.any(c1) or np.any(h1) or np.any(c2) or np.any(h2))

    packed_total = int(V[D])
    # compute tiles cover the packed rows only; the zx array is sized for the
    # waves' padded reads (rows past packed_total are never-consumed garbage)
    need = packed_total
    for d in range(1 if zero_init else 0, D):
        need = max(need, int(V[d]) + int(M[d]) * 128)
    Mzx = (packed_total + 127) // 128
    zx_row0 = int(V[1]) // 128 * 128 if (zero_init and D > 1) else 0
    zx_start_tile = zx_row0 // 128
    zx_rows = max(need - zx_row0, 128)

    depth = sch["depth"]; chain_id = sch["chain_id"]
    core_pos = sch["core_of_chain"][chain_id]
    packed_row = V[depth] + sch["core_rank"][chain_id]
    padded_row = P[depth] + sch["core_rank"][chain_id]

    x = np.ascontiguousarray(np.asarray(x, np.float32))
    xt_blocks = []
    for c in range(NCORES):
        sel = core_pos == c
        Xp = np.zeros((Mzx * 128, CIN), np.float32)
        Xp[packed_row[sel]] = x[sel]
        # lhsT block layout: [mt, p, k*128 + m] = Xp[mt*128+m, k*128+p]
        xt = Xp.reshape(Mzx, 128, KT, 128).transpose(0, 3, 2, 1).reshape(Mzx, 128, CIN)
        xt_blocks.append(np.ascontiguousarray(xt.astype(BF16)))

    # Wi in PE demand order: [half, k, p, g*512+c] = Wi[k*128+p, g*H+half*512+c]
    # so phase A's first m-tile consumes DMA chunks in arrival order.
    Wi_l = np.ascontiguousarray(
        np.asarray(Wi, np.float32).reshape(KT, 128, 4, 2, 512)
        .transpose(3, 0, 1, 2, 4).reshape(2, KT, 128, 2048).astype(BF16))
    # Wh in fp8 DoubleRow pair layout: [kp, p, i*G + n] = Wh[(2kp+i)*128+p, n]
    Wh_l = np.ascontiguousarray(
        np.asarray(Wh, np.float32).reshape(KT // 2, 2, 128, G)
        .transpose(0, 2, 1, 3).reshape(KT // 2, 128, 2 * G).astype(F8))
    bbc = np.ascontiguousarray(
        np.broadcast_to(np.asarray(b, np.float32)[None, :], (128, G)).astype(BF16))
    ident = np.ascontiguousarray(np.eye(128, dtype=np.float32).astype(BF16))

    ht0_blocks = [None] * NCORES
    c0_blocks = [None] * NCORES
    if not zero_init:
        h1 = np.asarray(h1, np.float32); h2 = np.asarray(h2, np.float32)
        c1 = np.asarray(c1, np.float32); c2 = np.asarray(c2, np.float32)
        hin = np.where(sch["chain_player"][:, None] > 0, h1[sch["chain_b"]],
                       h2[sch["chain_b"]])
        cin_ = np.where(sch["chain_player"][:, None] > 0, c1[sch["chain_b"]],
                        c2[sch["chain_b"]])
        live = sch["chain_seg"] == 0
        hin = np.where(live[:, None], hin, 0.0)
        cin_ = np.where(live[:, None], cin_, 0.0)
        M0 = int(M[0])
        for c in range(NCORES):
            selc = sch["core_of_chain"] == c
            rows = sch["core_rank"][selc]
            Hp = np.zeros((M0 * 128, H), np.float32)
            Cp = np.zeros((M0 * 128, H), np.float32)
            Hp[rows] = hin[selc]
            Cp[rows] = cin_[selc]
            # transposed layout for lhsT: [k, p, col] = Hp[col, k*128+p]
            ht0 = Hp.reshape(M0 * 128, KT, 128).transpose(1, 2, 0).astype(F8)
            ht0_blocks[c] = np.ascontiguousarray(ht0)
            c0_blocks[c] = np.ascontiguousarray(Cp)

    return dict(
        sch=sch, zero_init=zero_init, Mzx=Mzx, zx_row0=zx_row0, zx_rows=zx_rows,
        zx_start_tile=zx_start_tile, xt_blocks=xt_blocks, Wi_l=Wi_l, Wh_l=Wh_l,
        bbc=bbc, ident=ident, ht0_blocks=ht0_blocks, c0_blocks=c0_blocks,
        core_pos=core_pos, padded_row=padded_row, T=T, B=B,
    )


# ---------------------------------------------------------------------------
# Device program
# ---------------------------------------------------------------------------

def _build_program(D, U, M, V, P, Mzx, zx_row0, zx_rows, zx_start_tile,
                   zero_init, no_tail=False):
    nc = bacc.Bacc("TRN2", target_bir_lowering=False, debug=False)

    M0 = int(M[0])
    Ptot = int(P[D])
    d_start = 1 if zero_init else 0
    need_zx = D > d_start
    nzx_rows = zx_rows
    nwave1 = int(M[d_start]) if D > d_start else 0  # m-tiles of first wave

    xt_d = nc.dram_tensor("xt", [Mzx, 128, CIN], DT.bfloat16, kind="ExternalInput")
    wi_d = nc.dram_tensor("wi", [2, KT, 128, 2048], DT.bfloat16,
                          kind="ExternalInput")
    wh_d = nc.dram_tensor("wh", [KT // 2, 128, 2 * G], DT.float8e4,
                          kind="ExternalInput")
    bbc_d = nc.dram_tensor("bbc", [128, G], DT.bfloat16, kind="ExternalInput")
    id_d = nc.dram_tensor("ident", [128, 128], DT.bfloat16, kind="ExternalInput")
    y_d = nc.dram_tensor("y", [Ptot, H], DT.bfloat16, kind="ExternalOutput")
    if need_zx:
        zx_d = nc.dram_tensor("zx", [max(nzx_rows, 128), G], DT.bfloat16,
                              kind="Internal")
        zx_ap = zx_d.ap()
    hs_d = nc.dram_tensor("hstate", [Ptot, H], DT.bfloat16, kind="Internal")
    cs_d = nc.dram_tensor("cstate", [Ptot, H], DT.bfloat16, kind="Internal")
    if not zero_init:
        ht0_d = nc.dram_tensor("ht0", [KT, 128, M0 * 128], DT.float8e4,
                               kind="ExternalInput")
        c0_d = nc.dram_tensor("c0", [M0 * 128, H], DT.float32, kind="ExternalInput")

    xt_ap = xt_d.ap(); y_ap = y_d.ap(); hs_ap = hs_d.ap(); cs_ap = cs_d.ap()

    # Waves that keep state SBUF-resident: single m-tile, and the previous
    # wave small enough that its first m-tile's state tiles are still alive
    # (state-pool bufs=3 below).  M is monotone nonincreasing, so once a wave
    # qualifies every later wave does too.
    sbuf_in = [False] * D   # wave d takes h/c from prev wave's SBUF tiles
    for d in range(d_start + 1, D):
        if int(M[d]) == 1 and int(M[d - 1]) <= 3:
            sbuf_in[d] = True
    import os
    if no_tail or os.environ.get("KERNEL_NO_SBUF_TAIL"):
        sbuf_in = [False] * D

    # Wave d_start's h^T built during phase A from its per-tile hs stores
    w1_pre = zero_init and D > 1 and not no_tail \
        and not os.environ.get("KERNEL_NO_W1_PRE")

    with tile.TileContext(nc) as tc:
        from contextlib import ExitStack
        with ExitStack() as es:
            const = es.enter_context(tc.tile_pool(name="const", bufs=1))
            work = es.enter_context(tc.tile_pool(name="work", bufs=2))
            psum = es.enter_context(tc.tile_pool(name="psum", bufs=1, space="PSUM"))
            whPa = es.enter_context(tc.tile_pool(name="whPa", bufs=1))

            ident_sb = const.tile([128, 128], DT.bfloat16, tag="ident")
            # Wh lives entirely in SBUF as fp8 DoubleRow pairs (32KB/part),
            # prefetched during phase A: no phase-boundary weight loads.
            wh8_sb = whPa.tile([128, (KT // 2) * 2 * G], DT.float8e4,
                               tag="wh8", name="wh8_sb")
            if w1_pre:
                hT1P = es.enter_context(tc.tile_pool(name="hT1P", bufs=1))
                hT_w1 = hT1P.tile([128, KT * nwave1 * 128], DT.bfloat16,
                                  tag="hT1", name="hT_w1")
                hT_w1_f8 = hT1P.tile([128, KT * nwave1 * 128], DT.float8e4,
                                     tag="hT1f8", name="hT_w1_f8")

            def gate_math(z_src, c_src, out_row, store_h, store_c):
                """LSTM gate math for one 128-row m-tile.

                z_src: [128, G] bf16 SBUF tile holding z
                c_src: [128, H] fp32 ap with previous c, or None (c == 0;
                       the f gate is then unused and its sigmoid skipped).
                Returns (ncv, nhb) tiles (fp32 cell state, bf16 hidden).
                """
                gi = work.tile([128, H], DT.bfloat16, tag="gi", name="gi", bufs=1)
                gg = work.tile([128, H], DT.bfloat16, tag="gg", name="gg", bufs=1)
                go = work.tile([128, H], DT.bfloat16, tag="go", name="go", bufs=1)
                nc.scalar.activation(gi[:], z_src[:, 0 * H:1 * H], AF.Sigmoid)
                nc.scalar.activation(gg[:], z_src[:, 2 * H:3 * H], AF.Tanh)
                ncv = work.tile([128, H], DT.float32, tag="ncv", name="ncv",
                                bufs=3)
                if c_src is not None:
                    gf = work.tile([128, H], DT.bfloat16, tag="gf", name="gf",
                                   bufs=1)
                    nc.scalar.activation(gf[:], z_src[:, 1 * H:2 * H], AF.Sigmoid)
                    m1 = work.tile([128, H], DT.bfloat16, tag="f32a", name="m1",
                                   bufs=1)
                    nc.vector.tensor_mul(m1[:], gi[:], gg[:])
                    t1 = work.tile([128, H], DT.bfloat16, tag="f32b", name="t1",
                                   bufs=1)
                    nc.vector.tensor_mul(t1[:], gf[:], c_src[:])
                    nc.vector.tensor_add(ncv[:], t1[:], m1[:])
                else:
                    nc.vector.tensor_mul(ncv[:], gi[:], gg[:])
                nc.scalar.activation(go[:], z_src[:, 3 * H:4 * H], AF.Sigmoid)
                tnc = work.tile([128, H], DT.bfloat16, tag="f32b", name="tnc",
                                bufs=1)
                nc.scalar.activation(tnc[:], ncv[:], AF.Tanh)
                # nh produced directly in bf16 (y is stored bf16; host upcasts)
                nhb = work.tile([128, H], DT.bfloat16, tag="nhb", name="nhb",
                                bufs=3)
                nc.vector.tensor_mul(nhb[:], go[:], tnc[:])

                nc.gpsimd.dma_start(out=y_ap[out_row:out_row + 128, :], in_=nhb[:])
                if store_h:
                    nc.sync.dma_start(out=hs_ap[out_row:out_row + 128, :],
                                      in_=nhb[:])
                if store_c:
                    ncb = work.tile([128, H], DT.bfloat16, tag="ncb",
                                    name="ncb", bufs=2)
                    nc.vector.tensor_copy(ncb[:], ncv[:])
                    nc.sync.dma_start(out=cs_ap[out_row:out_row + 128, :],
                                      in_=ncb[:])
                return ncv, nhb

            def mm_half(lhsT_of_k, rhs_of_k, half, k_order=None):
                """Half an m-tile of z accumulation (512 of 1024 cols/gate).

                PSUM tags alternate by half (8 tags x 1 buf = all 8 banks), so
                one half's matmuls overlap the other half's drain, both within
                an m-tile and across m-tiles/waves.
                """
                pt = []
                for g in range(4):
                    pt.append(psum.tile([128, 512], DT.float32,
                                        tag=f"p{half}{g}", name=f"p{half}{g}",
                                        bufs=1))
                ks = list(k_order) if k_order is not None else list(range(KT))
                for i, k in enumerate(ks):
                    lhsT = lhsT_of_k(k)
                    for g in range(4):
                        col0 = g * H + half * 512
                        nc.tensor.matmul(
                            pt[g][:], lhsT=lhsT, rhs=rhs_of_k(k, col0),
                            start=(i == 0), stop=(i == len(ks) - 1),
                            skip_group_check=True)
                return pt

            # ---------------- phase A: zx = x@Wi + bias ----------------
            with tc.tile_pool(name="wiP", bufs=1) as wiP, \
                 tc.tile_pool(name="xtP", bufs=2) as xtP:
                # wi_sb layout matches the DRAM demand order:
                # [:, (half*KT + k)*2048 + g*512 + c]
                wi_sb = wiP.tile([128, KT * G], DT.bfloat16, tag="wi", name="wi_sb")
                # sync queue (HW DGE) carries the PE-critical startup loads in
                # exact consumption order; everything else goes via gpsimd
                xt0_sb = xtP.tile([128, CIN], DT.bfloat16, tag="xt", name="xt_sb")
                nc.sync.dma_start(out=xt0_sb[:], in_=xt_ap[0])
                # wi arrives in exact consumption order: the first half-0
                # k-slices ride the sync queue (parallel channel to gpsimd),
                # the rest stream per-slice via gpsimd
                for k in range(3):
                    off = k * 2048
                    nc.sync.dma_start(out=wi_sb[:, off:off + 2048],
                                      in_=wi_d.ap()[0][k])
                for k in range(3, KT):
                    off = k * 2048
                    nc.gpsimd.dma_start(out=wi_sb[:, off:off + 2048],
                                        in_=wi_d.ap()[0][k])
                h1off = KT * 2048
                nc.gpsimd.dma_start(
                    out=wi_sb[:, h1off:h1off + 4 * 2048].rearrange(
                        "p (k c) -> p k c", k=4),
                    in_=wi_d.ap()[1][0:4].rearrange("k p c -> p k c"))
                for k in range(4, KT):
                    off = h1off + k * 2048
                    nc.gpsimd.dma_start(out=wi_sb[:, off:off + 2048],
                                        in_=wi_d.ap()[1][k])
                bbc_sb = wiP.tile([128, G], DT.bfloat16, tag="bbc", name="bbc_sb")
                nc.sync.dma_start(out=bbc_sb[:], in_=bbc_d.ap()[:])
                nc.sync.dma_start(out=ident_sb[:], in_=id_d.ap()[:])

                def wi_rhs(k, col0):
                    g, within = divmod(col0, H)
                    off = ((within // 512) * KT + k) * 2048 + g * 512
                    return wi_sb[:, off:off + 512]

                # Tile emission order: wave-1-feeding prefix first (gate math
                # + h/c stores + per-tile h^T transpose chunks), then the
                # wave-zx tiles, then the remaining depth-0 tiles.  All wave
                # inputs are thus ready long before phase B starts.
                if zero_init and D > 1:
                    g1 = list(range(min(nwave1, Mzx)))
                    g2 = [t for t in range(zx_start_tile, Mzx) if t not in g1]
                    g3 = [t for t in range(nwave1, min(zx_start_tile, Mzx))]
                    order = g1 + g2 + g3
                else:
                    order = list(range(Mzx))
                wh_pref_at = min(nwave1 + 2, max(0, len(order) - 1))

                for idx, mt in enumerate(order):
                    if (idx == 2 and need_zx
                            and nzx_rows > Mzx * 128 - zx_row0):
                        # zero the zx rows past the packed region that deep
                        # waves read as padding: the identity-inject matmul
                        # would otherwise spread NaNs from uninitialized
                        # DRAM into every output row
                        gr0 = Mzx * 128 - zx_row0
                        zpad = work.tile([128, G], DT.bfloat16, tag="zws",
                                         name="zpad", bufs=3)
                        nc.vector.memset(zpad[:], 0)
                        nc.gpsimd.dma_start(out=zx_ap[gr0:nzx_rows, :],
                                            in_=zpad[0:nzx_rows - gr0, :])
                    if idx == wh_pref_at and (need_zx or D > 1):
                        for kp in range(KT // 2):
                            nc.gpsimd.dma_start(
                                out=wh8_sb[:, kp * 2 * G:(kp + 1) * 2 * G],
                                in_=wh_d.ap()[kp])
                    if mt == 0:
                        xt_sb = xt0_sb
                    else:
                        xt_sb = xtP.tile([128, CIN], DT.bfloat16, tag="xt",
                                         name="xt_sb")
                        eng = nc.sync if idx <= 2 else nc.gpsimd
                        eng.dma_start(out=xt_sb[:], in_=xt_ap[mt])

                    zxt = work.tile([128, G], DT.bfloat16, tag="zws", name="zxt",
                                    bufs=3)
                    for half in range(2):
                        pt = mm_half(
                            lambda k: xt_sb[:, k * 128:(k + 1) * 128],
                            wi_rhs, half)
                        # psum -> SBUF with bias add (bf16)
                        for g in range(4):
                            col0 = g * H + half * 512
                            nc.vector.tensor_add(zxt[:, col0:col0 + 512],
                                                 pt[g][:],
                                                 bbc_sb[:, col0:col0 + 512])

                    if need_zx and mt >= zx_start_tile:
                        r = mt * 128 - zx_row0
                        nc.gpsimd.dma_start(out=zx_ap[r:r + 128, :], in_=zxt[:])
                    if zero_init and mt < M0:
                        nwv = nwave1 if D > 1 else 0
                        st = D > 1 and mt < nwv
                        gate_math(zxt, None, mt * 128, store_h=st, store_c=st)
                        if w1_pre and mt < nwv:
                            # build wave-1's h^T chunk from the rows just
                            # stored; overlaps the remaining phase-A tiles
                            nc.sync.dma_start_transpose(
                                out=hT_w1.rearrange(
                                    "p (j m) -> p j m",
                                    m=nwave1 * 128)[:, :,
                                                    mt * 128:(mt + 1) * 128],
                                in_=hs_ap[mt * 128:(mt + 1) * 128, :])
                            if mt == nwv - 1:
                                nc.vector.tensor_copy(hT_w1_f8[:], hT_w1[:])

            # ---------------- phase B: waves ----------------
            if D > d_start:
                with tc.tile_pool(name="hTP", bufs=2) as hTP, \
                     tc.tile_pool(name="zxP", bufs=2) as zxP, \
                     tc.tile_pool(name="cP", bufs=2) as cP:
                    # pair kp covers k-slices (2kp, 2kp+1); pairs 2..3 are
                    # h-features 512..1023 (produced first by the half-1
                    # drain), so iterate them first
                    KP_ORDER = [2, 3, 0, 1]
                    NKP = KT // 2
                    DR = mybir.MatmulPerfMode.DoubleRow

                    def wh_rhs(kp, col0):
                        return wh8_sb[:, kp * 2 * G:(kp + 1) * 2 * G] \
                            .rearrange("p (i n) -> p i n", i=2)[
                                :, :, col0:col0 + 512]

                    def wave_mm(lhsT_of_kp, half, inject=None, at_i=None):
                        """One half's PSUM groups: optional bf16 zx inject
                        opener + 4 fp8 DoubleRow matmuls (K=256 each).
                        at_i(i) is called before pair i's matmuls (used to
                        slot the deferred h^T transposes into the PE queue
                        at the right point)."""
                        pt = []
                        for g in range(4):
                            pt.append(psum.tile([128, 512], DT.float32,
                                                tag=f"p{half}{g}",
                                                name=f"p{half}{g}", bufs=1))
                        if inject is not None:
                            for g in range(4):
                                col0 = g * H + half * 512
                                nc.tensor.matmul(
                                    pt[g][:], lhsT=ident_sb[:],
                                    rhs=inject[:, col0:col0 + 512],
                                    start=True, stop=False,
                                    skip_group_check=True)
                        for i, kp in enumerate(KP_ORDER):
                            if at_i is not None:
                                at_i(i)
                            for g in range(4):
                                col0 = g * H + half * 512
                                nc.tensor.matmul(
                                    pt[g][:], lhsT=lhsT_of_kp(kp),
                                    rhs=wh_rhs(kp, col0),
                                    start=(i == 0 and inject is None),
                                    stop=(i == NKP - 1),
                                    perf_mode=DR, skip_group_check=True)
                        return pt

                    if w1_pre:
                        dram_ds = [d for d in range(max(d_start, 1) + 1, D)
                                   if not sbuf_in[d]]
                        hT_cols_max = (KT * max(int(M[d]) for d in dram_ds) * 128
                                       if dram_ds else KT * 128)
                    else:
                        hT_cols_max = KT * int(M[max(d_start, 1)]) * 128 \
                            if D > max(d_start, 1) else KT * 128

                    def pair_view(hT8, ncols_, mt):
                        """[128, 2, 128] DoubleRow lhsT view of block pair
                        kp at m-tile mt of an fp8 h^T of ncols_ per block."""
                        v = hT8[:, 0:KT * ncols_].rearrange(
                            "p (j m) -> p j m", m=ncols_)
                        return lambda kp: v[:, 2 * kp:2 * kp + 2,
                                            mt * 128:mt * 128 + 128]

                    def emit_T(blocks, src_q, dst_hT):
                        """PE-transpose 128-col blocks of nh quarter tiles
                        into dst h^T (fp8 cast on the copy).  src_q maps
                        block index kk -> its [128, 256] quarter tile."""
                        for j, kk in enumerate(blocks):
                            htag = 1 if kk >= 4 else 0
                            ptT = psum.tile([128, 128], DT.bfloat16,
                                            tag=f"p{htag}{j}", name=f"ptT{kk}",
                                            bufs=1)
                            c0 = (kk % 2) * 128
                            nc.tensor.matmul(
                                ptT[:], lhsT=src_q[kk // 2][:, c0:c0 + 128],
                                rhs=ident_sb[:], is_transpose=True,
                                start=True, stop=True, skip_group_check=True)
                            nc.vector.tensor_copy(
                                dst_hT[:, kk * 128:(kk + 1) * 128], ptT[:])

                    prev_ncv = None
                    prev_nhb = None
                    pending_hT = None   # hT pre-built by the previous tail wave
                    pending_T0 = None   # deferred emission of hT blocks 0..3
                    pre_hT = None       # next DRAM wave's hT, built early
                    tail_zx = {}        # tail zx tiles prefetched a wave ahead

                    for d in range(d_start, D):
                        Md = int(M[d])
                        ncols = Md * 128

                        if sbuf_in[d]:
                            # ---- SBUF-resident tail wave (one m-tile) ----
                            if d in tail_zx:
                                zx_sb = tail_zx.pop(d)
                            else:
                                zx_sb = zxP.tile([128, G], DT.bfloat16,
                                                 tag="zx", name="zx_sb",
                                                 bufs=3)
                                r = int(V[d]) - zx_row0
                                nc.sync.dma_start(out=zx_sb[:],
                                                  in_=zx_ap[r:r + 128, :])
                            if d + 1 < D and sbuf_in[d + 1]:
                                nxt_zx = zxP.tile([128, G], DT.bfloat16,
                                                  tag="zx", name="zx_nxt",
                                                  bufs=3)
                                rn = int(V[d + 1]) - zx_row0
                                nc.sync.dma_start(out=nxt_zx[:],
                                                  in_=zx_ap[rn:rn + 128, :])
                                tail_zx[d + 1] = nxt_zx

                            if pending_hT is not None:
                                hT = pending_hT
                            else:
                                hTstage = hTP.tile([128, KT * 128],
                                                   DT.bfloat16, tag="hTsS",
                                                   name="hTstage")
                                nc.sync.dma_start_transpose(
                                    out=hTstage.rearrange("p (j c) -> p j c",
                                                          c=128),
                                    in_=prev_nhb[:])
                                hT = hTP.tile([128, KT * 128], DT.float8e4,
                                              tag="hTs", name="hTs")
                                nc.vector.tensor_copy(hT[:], hTstage[:])

                            build_next = d + 1 < D
                            if build_next:
                                hT_next = hTP.tile([128, KT * 128],
                                                   DT.float8e4, tag="hTs",
                                                   name="hTn")
                            ncv = work.tile([128, H], DT.float32, tag="ncv",
                                            name="ncv", bufs=3)
                            # nh in 256-col quarters: each feeds its h^T
                            # transposes the moment its subchain completes
                            nhq = [work.tile([128, 256], DT.bfloat16,
                                             tag=f"nhq{q}", name=f"nhq{q}",
                                             bufs=2) for q in range(4)]
                            lhsT_kp = pair_view(hT, 128, 0)

                            def tail_at_i(i):
                                nonlocal pending_T0
                                if i == 2 and pending_T0 is not None:
                                    pending_T0()
                                    pending_T0 = None

                            def tail_drain(half, pt):
                                for s in range(2):
                                    q = half * 2 + s
                                    sub = slice(s * 256, s * 256 + 256)
                                    blk = slice(half * 512 + s * 256,
                                                half * 512 + s * 256 + 256)
                                    gi = work.tile([128, 256], DT.bfloat16,
                                                   tag="gi", name="gi", bufs=1)
                                    gf = work.tile([128, 256], DT.bfloat16,
                                                   tag="gf", name="gf", bufs=1)
                                    gg = work.tile([128, 256], DT.bfloat16,
                                                   tag="gg", name="gg", bufs=1)
                                    go = work.tile([128, 256], DT.bfloat16,
                                                   tag="go", name="go", bufs=1)
                                    nc.scalar.activation(gf[:], pt[1][:, sub],
                                                         AF.Sigmoid)
                                    nc.scalar.activation(gi[:], pt[0][:, sub],
                                                         AF.Sigmoid)
                                    nc.scalar.activation(gg[:], pt[2][:, sub],
                                                         AF.Tanh)
                                    nc.scalar.activation(go[:], pt[3][:, sub],
                                                         AF.Sigmoid)
                                    t1b = work.tile([128, 256], DT.float32,
                                                    tag="t1b", name="t1b",
                                                    bufs=1)
                                    nc.vector.tensor_mul(t1b[:], gf[:],
                                                         prev_ncv[:, blk])
                                    m1b = work.tile([128, 256], DT.float32,
                                                    tag="m1b", name="m1b",
                                                    bufs=1)
                                    nc.vector.tensor_mul(m1b[:], gi[:], gg[:])
                                    nc.vector.tensor_add(ncv[:, blk], t1b[:],
                                                         m1b[:])
                                    tncb = work.tile([128, 256], DT.float32,
                                                     tag="tncb", name="tncb",
                                                     bufs=1)
                                    nc.scalar.activation(tncb[:], ncv[:, blk],
                                                         AF.Tanh)
                                    nc.vector.tensor_mul(nhq[q][:], go[:],
                                                         tncb[:])

                            # half 1 first: the next wave's matmuls consume
                            # feature pairs 2..3 first, built from this half.
                            # emit_T(4..7) sits between half 0's matmuls and
                            # its drain so the scalar queue runs the h^T
                            # copies before half 0's ACTs.
                            qmap = {2: nhq[2], 3: nhq[3], 0: nhq[0],
                                    1: nhq[1]}
                            pt1 = wave_mm(lhsT_kp, 1, inject=zx_sb,
                                          at_i=tail_at_i)
                            tail_drain(1, pt1)
                            nc.sync.dma_start(
                                out=y_ap[int(P[d]):int(P[d]) + 128, 512:768],
                                in_=nhq[2][:])
                            nc.sync.dma_start(
                                out=y_ap[int(P[d]):int(P[d]) + 128, 768:1024],
                                in_=nhq[3][:])
                            pt0 = wave_mm(lhsT_kp, 0, inject=zx_sb)
                            if build_next:
                                emit_T((4, 5, 6, 7), qmap, hT_next)
                            tail_drain(0, pt0)
                            nc.sync.dma_start(
                                out=y_ap[int(P[d]):int(P[d]) + 128, 0:256],
                                in_=nhq[0][:])
                            nc.sync.dma_start(
                                out=y_ap[int(P[d]):int(P[d]) + 128, 256:512],
                                in_=nhq[1][:])
                            if build_next:
                                # blocks 0..3 deferred into the next wave's
                                # pair loop so the PE never waits on the drain
                                pending_T0 = (lambda q_=qmap, hTn=hT_next:
                                              emit_T((0, 1, 2, 3), q_, hTn))

                            prev_ncv = ncv
                            pending_hT = hT_next if build_next else None
                            continue

                        # ---- DRAM-path wave ----
                        pending_hT = None
                        pending_T0 = None
                        if d == 0:
                            hT = None  # allocated per m-tile below
                        elif d == d_start and w1_pre:
                            hT = hT_w1_f8
                        elif pre_hT is not None:
                            hT = pre_hT  # transpose already ran mid-prev-wave
                        else:
                            hTstage = hTP.tile([128, hT_cols_max],
                                               DT.bfloat16, tag="hT",
                                               name="hTstage")
                            prev = int(P[d - 1])
                            nc.sync.dma_start_transpose(
                                out=hTstage[:, 0:KT * ncols].rearrange(
                                    "p (j m) -> p j m", m=ncols),
                                in_=hs_ap[prev:prev + ncols, :])
                            hT = hTP.tile([128, hT_cols_max], DT.float8e4,
                                          tag="hT8", name="hT8")
                            nc.vector.tensor_copy(hT[:, 0:KT * ncols],
                                                  hTstage[:, 0:KT * ncols])
                        pre_hT = None
                        prev_P = int(P[d - 1]) if d > 0 else 0
                        deferred = []
                        nxt_sbuf = (d + 1 < D) and sbuf_in[d + 1]
                        nwave = int(M[d + 1]) * 128 if d + 1 < D else 0

                        def drain_deferred(d=d, nxt_sbuf=nxt_sbuf,
                                           nwave=nwave):
                            nonlocal prev_ncv, prev_nhb, pre_hT
                            z_p, c_p, mt_p = deferred.pop(0)
                            ncv, nhb = gate_math(
                                z_p, c_p, int(P[d]) + mt_p * 128,
                                store_h=(d + 1 < D and not nxt_sbuf
                                         and mt_p * 128 < nwave),
                                store_c=(d + 1 < D and not nxt_sbuf
                                         and mt_p * 128 < nwave))
                            if mt_p == 0:
                                prev_ncv, prev_nhb = ncv, nhb
                            # once the next DRAM wave's source rows are all
                            # stored, run its h^T transpose (overlaps the
                            # remaining m-tiles)
                            if (d + 1 < D and not nxt_sbuf
                                    and mt_p == int(M[d + 1]) - 1):
                                ncols_n = int(M[d + 1]) * 128
                                preS = hTP.tile([128, hT_cols_max],
                                                DT.bfloat16, tag="hT",
                                                name="hTpreS")
                                nc.sync.dma_start_transpose(
                                    out=preS[:, 0:KT * ncols_n].rearrange(
                                        "p (j m) -> p j m", m=ncols_n),
                                    in_=hs_ap[int(P[d]):int(P[d]) + ncols_n,
                                              :])
                                pre_hT = hTP.tile([128, hT_cols_max],
                                                  DT.float8e4, tag="hT8",
                                                  name="hTpre")
                                nc.vector.tensor_copy(
                                    pre_hT[:, 0:KT * ncols_n],
                                    preS[:, 0:KT * ncols_n])

                        for mt in range(Md):
                            zx_sb = zxP.tile([128, G], DT.bfloat16, tag="zx",
                                             name="zx_sb", bufs=3)
                            r = int(V[d]) + mt * 128 - zx_row0
                            nc.gpsimd.dma_start(out=zx_sb[:],
                                                in_=zx_ap[r:r + 128, :])

                            if d == 0:
                                c_src = cP.tile([128, H], DT.float32, tag="c",
                                                name="c_sb")
                                nc.gpsimd.dma_start(
                                    out=c_src[:],
                                    in_=c0_d.ap()[mt * 128:mt * 128 + 128, :])
                            else:
                                c_src = cP.tile([128, H], DT.bfloat16,
                                                tag="c", name="c_sb")
                                nc.gpsimd.dma_start(
                                    out=c_src[:],
                                    in_=cs_ap[prev_P + mt * 128:
                                              prev_P + mt * 128 + 128, :])

                            if d == 0:
                                hT = hTP.tile([128, KT * 128], DT.float8e4,
                                              tag="hT0", name="hT0")
                                for k in range(KT):
                                    nc.gpsimd.dma_start(
                                        out=hT[:, k * 128:(k + 1) * 128],
                                        in_=ht0_d.ap()[k][:, mt * 128:
                                                          mt * 128 + 128])
                                lhsT_kp = pair_view(hT, 128, 0)
                            elif d == d_start and w1_pre:
                                lhsT_kp = pair_view(hT, nwave1 * 128, mt)
                            else:
                                lhsT_kp = pair_view(hT, ncols, mt)

                            z_sb = work.tile([128, G], DT.bfloat16, tag="zws",
                                             name="z_sb", bufs=3)
                            for half in range(2):
                                pt = wave_mm(lhsT_kp, half)
                                for g in range(4):
                                    col0 = g * H + half * 512
                                    nc.vector.tensor_add(
                                        z_sb[:, col0:col0 + 512], pt[g][:],
                                        zx_sb[:, col0:col0 + 512])

                            # defer the gate math by one m-tile: the z-adds
                            # (which free PSUM slots) stay at the head of the
                            # DVE queue, so the next tile's/wave's matmuls
                            # are never blocked behind trailing gate work
                            deferred.append((z_sb, c_src, mt))
                            # mt0 of the wave feeding the first SBUF tail
                            # wave must not be deferred: the tail's h^T xbar
                            # and prev_ncv consume it as early as possible
                            if len(deferred) > 1 or (mt == 0 and nxt_sbuf):
                                drain_deferred()

                        while deferred:
                            drain_deferred()

    nc.compile()
    return nc


# ---------------------------------------------------------------------------
# Entry point
# ---------------------------------------------------------------------------

_PROGRAM_CACHE = {}


def _run(inputs, trace=False):
    prep = _prep_inputs(**inputs)
    sch = prep["sch"]
    D, U, M, V, P = sch["D"], sch["U"], sch["M"], sch["V"], sch["P"]

    in_maps = []
    for c in range(NCORES):
        m = {
            "xt": prep["xt_blocks"][c],
            "wi": prep["Wi_l"],
            "wh": prep["Wh_l"],
            "bbc": prep["bbc"],
            "ident": prep["ident"],
        }
        if not prep["zero_init"]:
            m["ht0"] = prep["ht0_blocks"][c]
            m["c0"] = prep["c0_blocks"][c]
        in_maps.append(m)

    # Retry ladder: rare transient device errors have been observed on the
    # shared terminal; retry twice, then once more with the conservative
    # (no SBUF-resident tail waves) program variant.
    import time as _time
    res = None
    last_err = None
    for attempt, no_tail in enumerate([False, False, True]):
        key = (D, tuple(M.tolist()), tuple(U.tolist()), prep["Mzx"],
               prep["zx_row0"], prep["zx_rows"], prep["zero_init"], no_tail)
        try:
            if key not in _PROGRAM_CACHE:
                _PROGRAM_CACHE[key] = _build_program(
                    D, U, M, V, P, prep["Mzx"], prep["zx_row0"],
                    prep["zx_rows"], prep["zx_start_tile"], prep["zero_init"],
                    no_tail=no_tail)
            nc = _PROGRAM_CACHE[key]
            res = run_bass_kernel_spmd(nc, in_maps,
                                       core_ids=list(range(NCORES)),
                                       trace=trace)
            break
        except Exception as e:  # noqa: BLE001 - retry on device hiccups
            last_err = e
            sys.stderr.write(f"kernel attempt {attempt} failed: {e!r}\n")
            trace = False  # profiling hook may be wedged; drop it on retry
            _time.sleep(2.0)
    if res is None:
        raise last_err

    T, B = prep["T"], prep["B"]
    y_full = np.empty((T * B, H), np.float32)
    core_pos = prep["core_pos"]; padded_row = prep["padded_row"]
    for c in range(NCORES):
        sel = core_pos == c
        y_full[sel] = res.results[c]["y"][padded_row[sel]].astype(np.float32)
    return y_full, res


def kernel(**inputs) -> np.ndarray:
    y, _ = _run(inputs, trace=False)
    return y
